# revision 3
# baseline (speedup 1.0000x reference)
"""Trainium2 Bass kernel for causal multi-head attention (B=4, T=2048, C=1024, H=16).

Sharding: tensor-parallel over heads x batch. 8 cores = 4 batches x 2 head-halves.
Each core computes, for its batch b and its 8 heads (4 head-pairs):
  qkv projection -> causal attention -> output projection partial (rows of w_proj)
Host gathers by summing the two half-partials per batch (the "all-reduce").

v2 pipeline design (vs the phase-serial v1):
  - Everything bf16 (x, w_attn, w_proj): measured end-to-end rel err ~5e-3 vs
    the 2e-2 gate; halves input DMA + SBUF and enables fast weight load.
  - Single scheduling scope: projection t-chunks, attention q-chunks and the
    output projection are emitted interleaved (proj c0, attn q0, proj c1,
    oproj q0, attn q1, ...) so the Tile list-scheduler overlaps the ACT-bound
    softmax with PE-bound projection work instead of serializing phases.
  - Scores for a head PAIR run as two concurrent row-tiled matmuls
    (tile_size 64x128, tile_position (0,0)/(64,0)): contraction is d=64 so
    two heads share the 128x128 PE array -> 2x on the QK^T stream.
  - Scores land in one [128, 2{head}, 512] PSUM tile (two banks); exp for both
    heads is a single ACT instruction; the diagonal-block causal mask is one
    DVE multiply over both heads.
  - Softmax denominator: ones-column folded into V (row 64 of PV output);
    evacuated from PSUM by ACT (func=Copy shares the exp table), reciprocal'd
    on DVE after a DRAM-bounce partition-broadcast.
PSUM budget: proj/oproj accum 2 banks + scores 2x2 banks + PV pair 2 banks = 8.
"""

import sys

for _p in ("/opt/trn_rl_repo",):
    if _p not in sys.path:
        sys.path.insert(0, _p)

import numpy as np

import concourse.bass as bass
import concourse.mybir as mybir
import concourse.tile as tile
from concourse import bacc
from concourse.bass import ts
from concourse.bass_utils import run_bass_kernel_spmd

B, T, C, H, D = 4, 2048, 1024, 16, 64
NCORES = 8
JC = 512  # channels per core (8 heads x 64)
HL = 8  # heads per core
NP = 4  # head pairs per core
CT = C // 128  # 8 contraction tiles
NCH = T // 512  # 4 t/q chunks
F32 = mybir.dt.float32
BF16 = mybir.dt.bfloat16
EXP = mybir.ActivationFunctionType.Exp
CPY = mybir.ActivationFunctionType.Copy
ADD = mybir.AluOpType.add
MULT = mybir.AluOpType.mult


def _trace(nc, tc, io):
    xT, wq, wk, wv, wp, bq, bk, bv, bp, tri2, out = io

    with (
        tc.tile_pool(name="consts", bufs=1) as consts,
        tc.tile_pool(name="wts", bufs=1) as w_pool,
        tc.tile_pool(name="qk", bufs=1) as qk_pool,
        tc.tile_pool(name="vp", bufs=1) as v_pool,
        tc.tile_pool(name="yp", bufs=1) as y_pool,
        tc.tile_pool(name="xt", bufs=2) as xt_pool,
        tc.tile_pool(name="pp", bufs=2, space="PSUM") as ppsum,
        tc.tile_pool(name="sc", bufs=2, space="PSUM") as sc_pool,
        tc.tile_pool(name="pv", bufs=1, space="PSUM") as pv_pool,
        tc.tile_pool(name="pt", bufs=4) as pt_pool,
        tc.tile_pool(name="dn", bufs=2) as dn_pool,
        tc.tile_pool(name="rd", bufs=2) as rd_pool,
        tc.tile_pool(name="dsc", bufs=2, space="DRAM") as d_pool,
        tc.tile_pool(name="ob", bufs=2) as o_pool,
    ):
        # ---- weights + consts (wq and x chunk 0 first: they gate the
        # first projection matmuls; everything split per-ct so pieces
        # stream in across the DMA queues) ------------------------------
        xT_r = xT.rearrange("(ct p) t -> p ct t", p=128)
        wq_sb = w_pool.tile([128, CT, JC], BF16, tag="wq")
        wq_r = wq.rearrange("(ct p) j -> p ct j", p=128)
        xt0 = xt_pool.tile([128, CT, 512], BF16, tag="xt", name="xt0")
        for ct in range(CT):
            nc.sync.dma_start(out=wq_sb[:, ct, :], in_=wq_r[:, ct, :])
            nc.sync.dma_start(out=xt0[:, ct, :], in_=xT_r[:, ct, ts(0, 512)])
        wk_sb = w_pool.tile([128, CT, JC], BF16, tag="wk")
        wk_r = wk.rearrange("(ct p) j -> p ct j", p=128)
        wv_sb = w_pool.tile([128, CT, JC], BF16, tag="wv")
        wv_r = wv.rearrange("(ct p) j -> p ct j", p=128)
        for ct in range(CT):
            nc.sync.dma_start(out=wk_sb[:, ct, :], in_=wk_r[:, ct, :])
        for ct in range(CT):
            nc.sync.dma_start(out=wv_sb[:, ct, :], in_=wv_r[:, ct, :])
        tri_sb = consts.tile([128, 2, 128], BF16, tag="tri")
        nc.sync.dma_start(out=tri_sb, in_=tri2)
        bq_sb = consts.tile([128, 4], F32, tag="bq")
        nc.sync.dma_start(out=bq_sb, in_=bq.rearrange("(jt p) -> p jt", p=128))
        bk_sb = consts.tile([128, 4], F32, tag="bk")
        nc.sync.dma_start(out=bk_sb, in_=bk.rearrange("(jt p) -> p jt", p=128))
        bv_sb = consts.tile([128, JC], F32, tag="bv")
        nc.sync.dma_start(out=bv_sb, in_=bv.unsqueeze(0).to_broadcast([128, JC]))
        bp_sb = consts.tile([128, C], F32, tag="bp")
        nc.sync.dma_start(out=bp_sb, in_=bp.unsqueeze(0).to_broadcast([128, C]))
        wp_sb = w_pool.tile([128, 4, C], BF16, tag="wp")
        wp_r = wp.rearrange("(jt p) c -> p jt c", p=128)
        for jt in range(4):
            nc.sync.dma_start(out=wp_sb[:, jt, :], in_=wp_r[:, jt, :])

        q_sb = [
            qk_pool.tile([128, T], BF16, tag=f"q{jt}", name=f"q{jt}")
            for jt in range(NP)
        ]
        k_sb = [
            qk_pool.tile([128, T], BF16, tag=f"k{jt}", name=f"k{jt}")
            for jt in range(NP)
        ]
        v_sb = [
            v_pool.tile([128, HL, 65], BF16, tag=f"v{tt}", name=f"v{tt}")
            for tt in range(T // 128)
        ]
        y_sb = [
            y_pool.tile([128, T], BF16, tag=f"y{jt}", name=f"y{jt}")
            for jt in range(NP)
        ]

        def proj_chunk(c):
            if c == 0:
                xt_t = xt0
            else:
                xt_t = xt_pool.tile([128, CT, 512], BF16, tag="xt", name=f"xt{c}")
                for ct in range(CT):
                    nc.sync.dma_start(out=xt_t[:, ct, :], in_=xT_r[:, ct, ts(c, 512)])
            for wsb, bsb, dst in ((wq_sb, bq_sb, q_sb), (wk_sb, bk_sb, k_sb)):
                for jt in range(NP):
                    ps = ppsum.tile([128, 512], F32, tag="pp")
                    for ct in range(CT):
                        nc.tensor.matmul(
                            ps,
                            lhsT=wsb[:, ct, ts(jt, 128)],
                            rhs=xt_t[:, ct, :],
                            start=(ct == 0),
                            stop=(ct == CT - 1),
                        )
                    nc.vector.tensor_scalar_add(
                        out=dst[jt][:, ts(c, 512)], in0=ps, scalar1=bsb[:, jt : jt + 1]
                    )
            for sub in range(4):
                t128 = 4 * c + sub
                ps = ppsum.tile([128, 512], F32, tag="pp")
                for ct in range(CT):
                    nc.tensor.matmul(
                        ps,
                        lhsT=xt_t[:, ct, ts(sub, 128)],
                        rhs=wv_sb[:, ct, :],
                        start=(ct == 0),
                        stop=(ct == CT - 1),
                    )
                nc.vector.memset(v_sb[t128][:, :, 64:65], 1.0)
                nc.vector.tensor_tensor(
                    out=v_sb[t128][:, :, 0:64],
                    in0=ps.rearrange("p (h d) -> p h d", h=HL),
                    in1=bv_sb.rearrange("p (h d) -> p h d", h=HL),
                    op=ADD,
                )

        def attn_q(qc):
            for pr in range(NP):
                hA, hB = 2 * pr, 2 * pr + 1
                pv = pv_pool.tile([128, 2, 512], F32, tag="pv", name=f"pv{qc}_{pr}")
                lastkt = 4 * qc + 3
                for kt in range(lastkt + 1):
                    off = max(0, 128 * (kt - 4 * qc))
                    sc = sc_pool.tile(
                        [128, 2, 512], F32, tag="sc", name=f"sc{qc}_{pr}_{kt}"
                    )
                    nc.tensor.matmul(
                        sc[:, 0, off:512],
                        lhsT=k_sb[pr][0:64, ts(kt, 128)],
                        rhs=q_sb[pr][0:64, 512 * qc + off : 512 * (qc + 1)],
                        start=True,
                        stop=True,
                        tile_position=(0, 0),
                    )
                    nc.tensor.matmul(
                        sc[:, 1, off:512],
                        lhsT=k_sb[pr][64:128, ts(kt, 128)],
                        rhs=q_sb[pr][64:128, 512 * qc + off : 512 * (qc + 1)],
                        start=True,
                        stop=True,
                        tile_position=(64, 0),
                    )
                    pt = pt_pool.tile(
                        [128, 2, 512], BF16, tag="pt", name=f"pt{qc}_{pr}_{kt}"
                    )
                    nc.scalar.activation(
                        out=pt[:, :, off:512], in_=sc[:, :, off:512], func=EXP, scale=0.125
                    )
                    if kt >= 4 * qc:
                        # causal mask of the diagonal 128x128 block, both heads
                        nc.vector.tensor_tensor(
                            out=pt[:, :, off : off + 128],
                            in0=pt[:, :, off : off + 128],
                            in1=tri_sb,
                            op=MULT,
                        )
                    nc.tensor.matmul(
                        pv[0:65, 0, off:512],
                        lhsT=v_sb[kt][:, hA, :],
                        rhs=pt[:, 0, off:512],
                        start=(kt == 0),
                        stop=(kt == lastkt),
                    )
                    nc.tensor.matmul(
                        pv[0:65, 1, off:512],
                        lhsT=v_sb[kt][:, hB, :],
                        rhs=pt[:, 1, off:512],
                        start=(kt == 0),
                        stop=(kt == lastkt),
                    )
                den = dn_pool.tile([1, 2, 512], F32, tag="den", name=f"dn{qc}_{pr}")
                nc.scalar.activation(out=den, in_=pv[64:65, :, :], func=CPY)
                dscr = d_pool.tile([1024], F32, tag="dscr", name=f"ds{qc}_{pr}")
                nc.sync.dma_start(out=dscr.unsqueeze(0), in_=den)
                rdb = rd_pool.tile([64, 1024], F32, tag="rdb", name=f"rb{qc}_{pr}")
                nc.sync.dma_start(
                    out=rdb, in_=dscr.unsqueeze(0).to_broadcast([64, 1024])
                )
                rcp = rd_pool.tile([64, 1024], F32, tag="rcp", name=f"rc{qc}_{pr}")
                nc.vector.reciprocal_approx_fast(out=rcp, in_=rdb)
                for i in range(2):
                    nc.vector.tensor_tensor(
                        out=y_sb[pr][64 * i : 64 * i + 64, ts(qc, 512)],
                        in0=pv[0:64, i, :],
                        in1=rcp[:, ts(i, 512)],
                        op=MULT,
                    )

        def oproj_q(qc):
            for sub in range(4):
                t128 = 4 * qc + sub
                ot = o_pool.tile([128, C], F32, tag="ot", name=f"ot{t128}")
                for ch in range(2):
                    ps = ppsum.tile([128, 512], F32, tag="pp", name=f"op{t128}_{ch}")
                    for jt in range(NP):
                        nc.tensor.matmul(
                            ps,
                            lhsT=y_sb[jt][:, ts(t128, 128)],
                            rhs=wp_sb[:, jt, ts(ch, 512)],
                            start=(jt == 0),
                            stop=(jt == NP - 1),
                        )
                    nc.vector.tensor_tensor(
                        out=ot[:, ts(ch, 512)], in0=ps, in1=bp_sb[:, ts(ch, 512)], op=ADD
                    )
                nc.sync.dma_start(out=out[ts(t128, 128), :], in_=ot)

        # staircase: attn q-chunk c trails proj chunk c; oproj trails attn.
        # Emission order = scheduler priority: attention first so its scores
        # feed ACT early; projection matmuls fill the PE while ACT churns.
        proj_chunk(0)
        attn_q(0)
        proj_chunk(1)
        oproj_q(0)
        attn_q(1)
        proj_chunk(2)
        oproj_q(1)
        attn_q(2)
        proj_chunk(3)
        oproj_q(2)
        attn_q(3)
        oproj_q(3)


_CACHE = {}


def build_nc():
    if "nc" in _CACHE:
        return _CACHE["nc"]
    nc = bacc.Bacc(
        "TRN2",
        target_bir_lowering=False,
        debug=False,
        enable_asserts=False,
        num_devices=NCORES,
    )
    io = (
        nc.dram_tensor("xT", [C, T], BF16, kind="ExternalInput").ap(),
        nc.dram_tensor("wq", [C, JC], BF16, kind="ExternalInput").ap(),
        nc.dram_tensor("wk", [C, JC], BF16, kind="ExternalInput").ap(),
        nc.dram_tensor("wv", [C, JC], BF16, kind="ExternalInput").ap(),
        nc.dram_tensor("wp", [JC, C], BF16, kind="ExternalInput").ap(),
        nc.dram_tensor("bq", [JC], F32, kind="ExternalInput").ap(),
        nc.dram_tensor("bk", [JC], F32, kind="ExternalInput").ap(),
        nc.dram_tensor("bv", [JC], F32, kind="ExternalInput").ap(),
        nc.dram_tensor("bp", [C], F32, kind="ExternalInput").ap(),
        nc.dram_tensor("tri", [128, 2, 128], BF16, kind="ExternalInput").ap(),
        nc.dram_tensor("out", [T, C], F32, kind="ExternalOutput").ap(),
    )
    with tile.TileContext(nc) as tc:
        _trace(nc, tc, io)
    nc.compile()
    _CACHE["nc"] = nc
    return nc


def make_in_maps(x, w_attn, b_attn, w_proj, b_proj):
    import ml_dtypes

    BF = ml_dtypes.bfloat16
    tri = np.triu(np.ones((128, 128), dtype=np.float32))
    tri2 = np.ascontiguousarray(np.stack([tri, tri], axis=1)).astype(BF)
    zeros_c = np.zeros(C, dtype=np.float32)
    in_maps = []
    for core in range(NCORES):
        b, hh = core // 2, core % 2
        j0 = JC * hh
        in_maps.append(
            {
                "xT": np.ascontiguousarray(x[b].T).astype(BF),
                "wq": np.ascontiguousarray(w_attn[:, j0 : j0 + JC]).astype(BF),
                "wk": np.ascontiguousarray(w_attn[:, C + j0 : C + j0 + JC]).astype(BF),
                "wv": np.ascontiguousarray(
                    w_attn[:, 2 * C + j0 : 2 * C + j0 + JC]
                ).astype(BF),
                "wp": np.ascontiguousarray(w_proj[j0 : j0 + JC, :]).astype(BF),
                "bq": np.ascontiguousarray(b_attn[j0 : j0 + JC]).astype(np.float32),
                "bk": np.ascontiguousarray(b_attn[C + j0 : C + j0 + JC]).astype(
                    np.float32
                ),
                "bv": np.ascontiguousarray(b_attn[2 * C + j0 : 2 * C + j0 + JC]).astype(
                    np.float32
                ),
                "bp": (b_proj.astype(np.float32) if hh == 0 else zeros_c),
                "tri": tri2,
            }
        )
    return in_maps


def gather(parts):
    out = np.empty((B, T, C), dtype=np.float32)
    for b in range(B):
        out[b] = parts[2 * b]["out"] + parts[2 * b + 1]["out"]
    return out


def kernel(x, w_attn, b_attn, w_proj, b_proj):
    x = np.asarray(x, dtype=np.float32)
    w_attn = np.asarray(w_attn, dtype=np.float32)
    b_attn = np.asarray(b_attn, dtype=np.float32)
    w_proj = np.asarray(w_proj, dtype=np.float32)
    b_proj = np.asarray(b_proj, dtype=np.float32)
    nc = build_nc()
    in_maps = make_in_maps(x, w_attn, b_attn, w_proj, b_proj)
    res = run_bass_kernel_spmd(nc, in_maps, core_ids=list(range(NCORES)))
    return gather(res.results)


if __name__ == "__main__":
    rng = np.random.default_rng(0)
    x = rng.standard_normal((B, T, C), dtype=np.float32)
    w_attn = rng.standard_normal((C, 3 * C), dtype=np.float32) / np.sqrt(C)
    b_attn = np.zeros(3 * C, np.float32)
    w_proj = rng.standard_normal((C, C), dtype=np.float32) / np.sqrt(C)
    b_proj = np.zeros(C, np.float32)
    out = kernel(x, w_attn, b_attn, w_proj, b_proj)
    print(out.shape, out.dtype, np.abs(out).mean())


# revision 5
# speedup vs baseline: 1.2692x; 1.2692x over previous
"""Trainium2 Bass kernel for causal multi-head attention (B=4, T=2048, C=1024, H=16).

Sharding: tensor-parallel over heads x batch. 8 cores = 4 batches x 2 head-halves.
Each core computes, for its batch b and its 8 heads (4 head-pairs):
  qkv projection -> causal attention -> output projection partial (rows of w_proj)
Host gathers by summing the two half-partials per batch (the "all-reduce").

v2 pipeline design (vs the phase-serial v1):
  - Everything bf16 (x, w_attn, w_proj): measured end-to-end rel err ~5e-3 vs
    the 2e-2 gate; halves input DMA + SBUF and enables fast weight load.
  - Single scheduling scope: projection t-chunks, attention q-chunks and the
    output projection are emitted interleaved (proj c0, attn q0, proj c1,
    oproj q0, attn q1, ...) so the Tile list-scheduler overlaps the ACT-bound
    softmax with PE-bound projection work instead of serializing phases.
  - Scores for a head PAIR run as two concurrent row-tiled matmuls
    (tile_size 64x128, tile_position (0,0)/(64,0)): contraction is d=64 so
    two heads share the 128x128 PE array -> 2x on the QK^T stream.
  - Scores land in one [128, 2{head}, 512] PSUM tile (two banks); exp for both
    heads is a single ACT instruction; the diagonal-block causal mask is one
    DVE multiply over both heads.
  - Softmax denominator: ones-column folded into V (row 64 of PV output);
    evacuated from PSUM by ACT (func=Copy shares the exp table), reciprocal'd
    on DVE after a DRAM-bounce partition-broadcast.
PSUM budget: proj/oproj accum 2 banks + scores 2x2 banks + PV pair 2 banks = 8.
"""

import sys

for _p in ("/opt/trn_rl_repo",):
    if _p not in sys.path:
        sys.path.insert(0, _p)

import numpy as np

import concourse.bass as bass
import concourse.mybir as mybir
import concourse.tile as tile
from concourse import bacc
from concourse.bass import ts
from concourse.bass_utils import run_bass_kernel_spmd

B, T, C, H, D = 4, 2048, 1024, 16, 64
NCORES = 8
JC = 512  # channels per core (8 heads x 64)
HL = 8  # heads per core
NP = 4  # head pairs per core
CT = C // 128  # 8 contraction tiles
NCH = T // 512  # 4 t/q chunks
F32 = mybir.dt.float32
BF16 = mybir.dt.bfloat16
EXP = mybir.ActivationFunctionType.Exp
CPY = mybir.ActivationFunctionType.Copy
ADD = mybir.AluOpType.add
MULT = mybir.AluOpType.mult


def _trace(nc, tc, io):
    xT, wq, wk, wv, wp, bq, bk, bv, bp, tri2, out = io

    with (
        tc.tile_pool(name="consts", bufs=1) as consts,
        tc.tile_pool(name="wts", bufs=1) as w_pool,
        tc.tile_pool(name="qk", bufs=1) as qk_pool,
        tc.tile_pool(name="vp", bufs=1) as v_pool,
        tc.tile_pool(name="yp", bufs=1) as y_pool,
        tc.tile_pool(name="xt", bufs=2) as xt_pool,
        tc.tile_pool(name="pp", bufs=2, space="PSUM") as ppsum,
        tc.tile_pool(name="sc", bufs=2, space="PSUM") as sc_pool,
        tc.tile_pool(name="pv", bufs=1, space="PSUM") as pv_pool,
        tc.tile_pool(name="pt", bufs=4) as pt_pool,
        tc.tile_pool(name="pvs", bufs=2) as pvs_pool,
        tc.tile_pool(name="rd", bufs=2) as rd_pool,
        tc.tile_pool(name="dsc", bufs=2, space="DRAM") as d_pool,
        tc.tile_pool(name="ob", bufs=2) as o_pool,
    ):
        # ---- weights + consts (wq and x chunk 0 first: they gate the
        # first projection matmuls; everything split per-ct so pieces
        # stream in across the DMA queues) ------------------------------
        xT_r = xT.rearrange("(ct p) t -> p ct t", p=128)
        wq_sb = w_pool.tile([128, CT, JC], BF16, tag="wq")
        wq_r = wq.rearrange("(ct p) j -> p ct j", p=128)
        xt0 = xt_pool.tile([128, CT, 512], BF16, tag="xt", name="xt0")
        for ct in range(CT):
            nc.sync.dma_start(out=wq_sb[:, ct, :], in_=wq_r[:, ct, :])
            nc.sync.dma_start(out=xt0[:, ct, :], in_=xT_r[:, ct, ts(0, 512)])
        wk_sb = w_pool.tile([128, CT, JC], BF16, tag="wk")
        wk_r = wk.rearrange("(ct p) j -> p ct j", p=128)
        wv_sb = w_pool.tile([128, CT, JC], BF16, tag="wv")
        wv_r = wv.rearrange("(ct p) j -> p ct j", p=128)
        for ct in range(CT):
            nc.sync.dma_start(out=wk_sb[:, ct, :], in_=wk_r[:, ct, :])
        for ct in range(CT):
            nc.sync.dma_start(out=wv_sb[:, ct, :], in_=wv_r[:, ct, :])
        tri_sb = consts.tile([128, 2, 128], BF16, tag="tri")
        nc.sync.dma_start(out=tri_sb, in_=tri2)
        bq_sb = consts.tile([128, 4], F32, tag="bq")
        nc.sync.dma_start(out=bq_sb, in_=bq.rearrange("(jt p) -> p jt", p=128))
        bk_sb = consts.tile([128, 4], F32, tag="bk")
        nc.sync.dma_start(out=bk_sb, in_=bk.rearrange("(jt p) -> p jt", p=128))
        bv_sb = consts.tile([128, JC], F32, tag="bv")
        nc.sync.dma_start(out=bv_sb, in_=bv.unsqueeze(0).to_broadcast([128, JC]))
        bp_sb = consts.tile([128, C], F32, tag="bp")
        nc.sync.dma_start(out=bp_sb, in_=bp.unsqueeze(0).to_broadcast([128, C]))
        wp_sb = w_pool.tile([128, 4, C], BF16, tag="wp")
        wp_r = wp.rearrange("(jt p) c -> p jt c", p=128)
        for jt in range(4):
            nc.sync.dma_start(out=wp_sb[:, jt, :], in_=wp_r[:, jt, :])

        q_sb = [
            qk_pool.tile([128, T], BF16, tag=f"q{jt}", name=f"q{jt}")
            for jt in range(NP)
        ]
        k_sb = [
            qk_pool.tile([128, T], BF16, tag=f"k{jt}", name=f"k{jt}")
            for jt in range(NP)
        ]
        v_sb = [
            v_pool.tile([128, HL, 65], BF16, tag=f"v{tt}", name=f"v{tt}")
            for tt in range(T // 128)
        ]
        y_sb = [
            y_pool.tile([128, T], BF16, tag=f"y{jt}", name=f"y{jt}")
            for jt in range(NP)
        ]

        def proj_chunk(c):
            if c == 0:
                xt_t = xt0
            else:
                xt_t = xt_pool.tile([128, CT, 512], BF16, tag="xt", name=f"xt{c}")
                for ct in range(CT):
                    nc.sync.dma_start(out=xt_t[:, ct, :], in_=xT_r[:, ct, ts(c, 512)])
            for wsb, bsb, dst in ((wq_sb, bq_sb, q_sb), (wk_sb, bk_sb, k_sb)):
                for jt in range(NP):
                    ps = ppsum.tile([128, 512], F32, tag="pp")
                    for ct in range(CT):
                        nc.tensor.matmul(
                            ps,
                            lhsT=wsb[:, ct, ts(jt, 128)],
                            rhs=xt_t[:, ct, :],
                            start=(ct == 0),
                            stop=(ct == CT - 1),
                        )
                    nc.vector.tensor_scalar_add(
                        out=dst[jt][:, ts(c, 512)], in0=ps, scalar1=bsb[:, jt : jt + 1]
                    )
            for sub in range(4):
                t128 = 4 * c + sub
                ps = ppsum.tile([128, 512], F32, tag="pp")
                for ct in range(CT):
                    nc.tensor.matmul(
                        ps,
                        lhsT=xt_t[:, ct, ts(sub, 128)],
                        rhs=wv_sb[:, ct, :],
                        start=(ct == 0),
                        stop=(ct == CT - 1),
                    )
                nc.vector.memset(v_sb[t128][:, :, 64:65], 1.0)
                nc.vector.tensor_tensor(
                    out=v_sb[t128][:, :, 0:64],
                    in0=ps.rearrange("p (h d) -> p h d", h=HL),
                    in1=bv_sb.rearrange("p (h d) -> p h d", h=HL),
                    op=ADD,
                )

        def attn_q(qc):
            for pr in range(NP):
                hA, hB = 2 * pr, 2 * pr + 1
                pv = pv_pool.tile([128, 2, 512], F32, tag="pv", name=f"pv{qc}_{pr}")
                lastkt = 4 * qc + 3
                for kt in range(lastkt + 1):
                    off = max(0, 128 * (kt - 4 * qc))
                    sc = sc_pool.tile(
                        [128, 2, 512], F32, tag="sc", name=f"sc{qc}_{pr}_{kt}"
                    )
                    nc.tensor.matmul(
                        sc[:, 0, off:512],
                        lhsT=k_sb[pr][0:64, ts(kt, 128)],
                        rhs=q_sb[pr][0:64, 512 * qc + off : 512 * (qc + 1)],
                        start=True,
                        stop=True,
                        tile_position=(0, 0),
                    )
                    nc.tensor.matmul(
                        sc[:, 1, off:512],
                        lhsT=k_sb[pr][64:128, ts(kt, 128)],
                        rhs=q_sb[pr][64:128, 512 * qc + off : 512 * (qc + 1)],
                        start=True,
                        stop=True,
                        tile_position=(64, 0),
                    )
                    pt = pt_pool.tile(
                        [128, 2, 512], BF16, tag="pt", name=f"pt{qc}_{pr}_{kt}"
                    )
                    nc.scalar.activation(
                        out=pt[:, :, off:512], in_=sc[:, :, off:512], func=EXP, scale=0.125
                    )
                    if kt >= 4 * qc:
                        # causal mask of the diagonal 128x128 block, both heads
                        nc.vector.tensor_tensor(
                            out=pt[:, :, off : off + 128],
                            in0=pt[:, :, off : off + 128],
                            in1=tri_sb,
                            op=MULT,
                        )
                    nc.tensor.matmul(
                        pv[0:65, 0, off:512],
                        lhsT=v_sb[kt][:, hA, :],
                        rhs=pt[:, 0, off:512],
                        start=(kt == 0),
                        stop=(kt == lastkt),
                    )
                    nc.tensor.matmul(
                        pv[0:65, 1, off:512],
                        lhsT=v_sb[kt][:, hB, :],
                        rhs=pt[:, 1, off:512],
                        start=(kt == 0),
                        stop=(kt == lastkt),
                    )
                # evacuate PV psum in one DVE copy so the bank frees fast;
                # the denominator DMA-bounce chain then runs off SBUF,
                # off the Tensor engine's critical path.
                pvs = pvs_pool.tile([65, 2, 512], F32, tag="pvs", name=f"pvs{qc}_{pr}")
                nc.vector.tensor_copy(out=pvs, in_=pv[0:65, :, :])
                dscr = d_pool.tile([1024], F32, tag="dscr", name=f"ds{qc}_{pr}")
                nc.sync.dma_start(out=dscr.unsqueeze(0), in_=pvs[64:65, :, :])
                rdb = rd_pool.tile([64, 1024], F32, tag="rdb", name=f"rb{qc}_{pr}")
                nc.sync.dma_start(
                    out=rdb, in_=dscr.unsqueeze(0).to_broadcast([64, 1024])
                )
                rcp = rd_pool.tile([64, 1024], F32, tag="rcp", name=f"rc{qc}_{pr}")
                nc.vector.reciprocal_approx_fast(out=rcp, in_=rdb)
                for i in range(2):
                    nc.vector.tensor_tensor(
                        out=y_sb[pr][64 * i : 64 * i + 64, ts(qc, 512)],
                        in0=pvs[0:64, i, :],
                        in1=rcp[:, ts(i, 512)],
                        op=MULT,
                    )

        def oproj_q(qc):
            for sub in range(4):
                t128 = 4 * qc + sub
                ot = o_pool.tile([128, C], F32, tag="ot", name=f"ot{t128}")
                for ch in range(2):
                    ps = ppsum.tile([128, 512], F32, tag="pp", name=f"op{t128}_{ch}")
                    for jt in range(NP):
                        nc.tensor.matmul(
                            ps,
                            lhsT=y_sb[jt][:, ts(t128, 128)],
                            rhs=wp_sb[:, jt, ts(ch, 512)],
                            start=(jt == 0),
                            stop=(jt == NP - 1),
                        )
                    nc.vector.tensor_tensor(
                        out=ot[:, ts(ch, 512)], in0=ps, in1=bp_sb[:, ts(ch, 512)], op=ADD
                    )
                nc.sync.dma_start(out=out[ts(t128, 128), :], in_=ot)

        # staircase: attn q-chunk c trails proj chunk c; oproj trails attn.
        # Emission order = scheduler priority: attention first so its scores
        # feed ACT early; projection matmuls fill the PE while ACT churns.
        proj_chunk(0)
        attn_q(0)
        proj_chunk(1)
        oproj_q(0)
        attn_q(1)
        proj_chunk(2)
        oproj_q(1)
        attn_q(2)
        proj_chunk(3)
        oproj_q(2)
        attn_q(3)
        oproj_q(3)


_CACHE = {}


def build_nc():
    if "nc" in _CACHE:
        return _CACHE["nc"]
    nc = bacc.Bacc(
        "TRN2",
        target_bir_lowering=False,
        debug=False,
        enable_asserts=False,
        num_devices=NCORES,
    )
    io = (
        nc.dram_tensor("xT", [C, T], BF16, kind="ExternalInput").ap(),
        nc.dram_tensor("wq", [C, JC], BF16, kind="ExternalInput").ap(),
        nc.dram_tensor("wk", [C, JC], BF16, kind="ExternalInput").ap(),
        nc.dram_tensor("wv", [C, JC], BF16, kind="ExternalInput").ap(),
        nc.dram_tensor("wp", [JC, C], BF16, kind="ExternalInput").ap(),
        nc.dram_tensor("bq", [JC], F32, kind="ExternalInput").ap(),
        nc.dram_tensor("bk", [JC], F32, kind="ExternalInput").ap(),
        nc.dram_tensor("bv", [JC], F32, kind="ExternalInput").ap(),
        nc.dram_tensor("bp", [C], F32, kind="ExternalInput").ap(),
        nc.dram_tensor("tri", [128, 2, 128], BF16, kind="ExternalInput").ap(),
        nc.dram_tensor("out", [T, C], F32, kind="ExternalOutput").ap(),
    )
    with tile.TileContext(nc) as tc:
        _trace(nc, tc, io)
    nc.compile()
    _CACHE["nc"] = nc
    return nc


def make_in_maps(x, w_attn, b_attn, w_proj, b_proj):
    import ml_dtypes

    BF = ml_dtypes.bfloat16
    tri = np.triu(np.ones((128, 128), dtype=np.float32))
    tri2 = np.ascontiguousarray(np.stack([tri, tri], axis=1)).astype(BF)
    zeros_c = np.zeros(C, dtype=np.float32)
    in_maps = []
    for core in range(NCORES):
        b, hh = core // 2, core % 2
        j0 = JC * hh
        in_maps.append(
            {
                "xT": np.ascontiguousarray(x[b].T).astype(BF),
                "wq": np.ascontiguousarray(w_attn[:, j0 : j0 + JC]).astype(BF),
                "wk": np.ascontiguousarray(w_attn[:, C + j0 : C + j0 + JC]).astype(BF),
                "wv": np.ascontiguousarray(
                    w_attn[:, 2 * C + j0 : 2 * C + j0 + JC]
                ).astype(BF),
                "wp": np.ascontiguousarray(w_proj[j0 : j0 + JC, :]).astype(BF),
                "bq": np.ascontiguousarray(b_attn[j0 : j0 + JC]).astype(np.float32),
                "bk": np.ascontiguousarray(b_attn[C + j0 : C + j0 + JC]).astype(
                    np.float32
                ),
                "bv": np.ascontiguousarray(b_attn[2 * C + j0 : 2 * C + j0 + JC]).astype(
                    np.float32
                ),
                "bp": (b_proj.astype(np.float32) if hh == 0 else zeros_c),
                "tri": tri2,
            }
        )
    return in_maps


def gather(parts):
    out = np.empty((B, T, C), dtype=np.float32)
    for b in range(B):
        out[b] = parts[2 * b]["out"] + parts[2 * b + 1]["out"]
    return out


def kernel(x, w_attn, b_attn, w_proj, b_proj):
    x = np.asarray(x, dtype=np.float32)
    w_attn = np.asarray(w_attn, dtype=np.float32)
    b_attn = np.asarray(b_attn, dtype=np.float32)
    w_proj = np.asarray(w_proj, dtype=np.float32)
    b_proj = np.asarray(b_proj, dtype=np.float32)
    nc = build_nc()
    in_maps = make_in_maps(x, w_attn, b_attn, w_proj, b_proj)
    res = run_bass_kernel_spmd(nc, in_maps, core_ids=list(range(NCORES)))
    return gather(res.results)


if __name__ == "__main__":
    rng = np.random.default_rng(0)
    x = rng.standard_normal((B, T, C), dtype=np.float32)
    w_attn = rng.standard_normal((C, 3 * C), dtype=np.float32) / np.sqrt(C)
    b_attn = np.zeros(3 * C, np.float32)
    w_proj = rng.standard_normal((C, C), dtype=np.float32) / np.sqrt(C)
    b_proj = np.zeros(C, np.float32)
    out = kernel(x, w_attn, b_attn, w_proj, b_proj)
    print(out.shape, out.dtype, np.abs(out).mean())
